# revision 6
# baseline (speedup 1.0000x reference)
"""BigBird attention kernel for 8 Trainium2 NeuronCores.

Sharding: data-parallel over batch (2) x tensor-parallel over heads (4 groups
of 4 heads) = 8 cores. Each core computes q/k/v projections for its head
slice, block-sparse masked attention (128x128 supertiles derived from the
runtime mask), and a partial output projection with its Wo row-slice. The
host sums the 4 partial outputs per batch.
"""

import sys

for _p in ("/opt/trn_rl_repo", "/opt/trn_rl_repo/concourse"):
    if _p not in sys.path:
        sys.path.insert(0, _p)

import numpy as np

import concourse.bacc as bacc
import concourse.bass as bass
import concourse.mybir as mybir
import concourse.tile as tile
from concourse import bass_utils

F32 = mybir.dt.float32

B, S, D, H = 2, 2048, 1024, 16
HD = D // H          # 64
SCALE = 1.0 / float(np.sqrt(HD))
NCORES = 8
HG = 4               # head groups (tensor-parallel)
HPC = H // HG        # heads per core = 4
DC = HPC * HD        # channels per core = 256
QT = 128             # supertile edge
NQ = S // QT         # 16
NK = S // QT         # 16
VW = HD + 1          # v columns per head incl. ones column


def _mask_pattern(mask):
    """Derive the block-sparse schedule from the runtime mask."""
    sup = mask.reshape(NQ, QT, NK, QT).any(axis=(1, 3))  # [16,16]
    kts = [np.nonzero(sup[qi])[0].tolist() for qi in range(NQ)]
    cnts = [len(k) for k in kts]
    maxw = max(max(cnts), 1) * QT
    # segments: consecutive kt runs, split so each scores matmul stays inside
    # one psum bank (4 slots of 128 = 512 fp32, the fp32 moving-N limit too)
    segs = []
    for qi in range(NQ):
        s = []
        slot = 0
        while slot < cnts[qi]:
            start = slot
            while (
                slot + 1 < cnts[qi]
                and kts[qi][slot + 1] == kts[qi][slot] + 1
                and (slot + 1) // 4 == start // 4
            ):
                slot += 1
            s.append((start, kts[qi][start], slot - start + 1))
            slot += 1
        segs.append(s)
    return kts, cnts, segs, maxw


def _build_nc(kts, cnts, segs, maxw):
    nc = bacc.Bacc("TRN2", target_bir_lowering=False, debug=False)

    xT_d = nc.dram_tensor("xT", [D, S], F32, kind="ExternalInput")
    wq_d = nc.dram_tensor("wq", [D, DC], F32, kind="ExternalInput")
    wk_d = nc.dram_tensor("wk", [D, DC], F32, kind="ExternalInput")
    wv_d = nc.dram_tensor("wv", [D, DC], F32, kind="ExternalInput")
    wo_d = nc.dram_tensor("wo", [DC, D], F32, kind="ExternalInput")
    cos_d = nc.dram_tensor("cosT", [128, S], F32, kind="ExternalInput")
    sin_d = nc.dram_tensor("sinT", [128, S], F32, kind="ExternalInput")
    rt_d = nc.dram_tensor("rT", [128, 128], F32, kind="ExternalInput")
    id_d = nc.dram_tensor("ident", [128, 128], F32, kind="ExternalInput")
    mk_d = nc.dram_tensor("maskc", [NQ, QT, maxw], F32, kind="ExternalInput")
    out_d = nc.dram_tensor("out", [S, D], F32, kind="ExternalOutput")

    KC = D // 128   # 8 contraction chunks
    CC = DC // 128  # 2 channel chunks (2 heads each)

    with tile.TileContext(nc) as tc:
        with (
            tc.tile_pool(name="persist", bufs=1) as pp,
            tc.tile_pool(name="wop", bufs=1) as wop,
        ):
            # persistent sbuf tensors
            qrT = [pp.tile([128, S], F32, tag=f"qrT{c}", name=f"qrT{c}") for c in range(CC)]
            krT = [pp.tile([128, S], F32, tag=f"krT{c}", name=f"krT{c}") for c in range(CC)]
            v_sb = [pp.tile([128, HPC * VW], F32, tag=f"v{i}", name=f"v{i}") for i in range(NQ)]
            otT = [pp.tile([128, S], F32, tag=f"otT{c}", name=f"otT{c}") for c in range(CC)]
            ident = pp.tile([128, 128], F32, tag="ident")
            wo_sb = [wop.tile([128, D], F32, tag=f"wo{c}", name=f"wo{c}") for c in range(CC)]
            nc.sync.dma_start(ident[:], id_d[:, :])
            for c in range(CC):
                nc.sync.dma_start(wo_sb[c][:], wo_d[c * 128:(c + 1) * 128, :])

            # ---------------- QKV + RoPE ----------------
            with (
                tc.tile_pool(name="qkv_in", bufs=1) as qp,
                tc.tile_pool(name="qkv_scr", bufs=3) as sp,
                tc.tile_pool(name="qkv_ps", bufs=2, space="PSUM") as psp,
                tc.tile_pool(name="qkv_psv", bufs=2, space="PSUM") as psv,
            ):
                xT = [qp.tile([128, S], F32, tag=f"xT{k}", name=f"xT{k}") for k in range(KC)]
                wq_sb = [qp.tile([128, DC], F32, tag=f"wq{k}", name=f"wq{k}") for k in range(KC)]
                wk_sb = [qp.tile([128, DC], F32, tag=f"wk{k}", name=f"wk{k}") for k in range(KC)]
                wv_sb = [qp.tile([128, DC], F32, tag=f"wv{k}", name=f"wv{k}") for k in range(KC)]
                cosT = qp.tile([128, S], F32, tag="cosT")
                sinT = qp.tile([128, S], F32, tag="sinT")
                rT = qp.tile([128, 128], F32, tag="rT")
                for k in range(KC):
                    nc.sync.dma_start(xT[k][:], xT_d[k * 128:(k + 1) * 128, :])
                    nc.sync.dma_start(wq_sb[k][:], wq_d[k * 128:(k + 1) * 128, :])
                    nc.sync.dma_start(wk_sb[k][:], wk_d[k * 128:(k + 1) * 128, :])
                    nc.sync.dma_start(wv_sb[k][:], wv_d[k * 128:(k + 1) * 128, :])
                nc.sync.dma_start(cosT[:], cos_d[:, :])
                nc.sync.dma_start(sinT[:], sin_d[:, :])
                nc.sync.dma_start(rT[:], rt_d[:, :])

                # q^T / k^T with rope applied in-place
                for cc in range(CC):
                    for pc in range(S // 512):
                        fs = slice(pc * 512, (pc + 1) * 512)
                        for w_sb, dstT, tg in (
                            (wq_sb, qrT, "q"),
                            (wk_sb, krT, "k"),
                        ):
                            ps = psp.tile([128, 512], F32, tag=f"ps_{tg}", name=f"ps_{tg}")
                            for k in range(KC):
                                nc.tensor.matmul(
                                    ps[:],
                                    w_sb[k][:, cc * 128:(cc + 1) * 128],
                                    xT[k][:, fs],
                                    start=(k == 0),
                                    stop=(k == KC - 1),
                                )
                            raw = sp.tile([128, 512], F32, tag="raw")
                            nc.vector.tensor_copy(raw[:], ps[:])
                            rot = psp.tile([128, 512], F32, tag="rot")
                            nc.tensor.matmul(
                                rot[:], rT[:], raw[:], start=True, stop=True
                            )
                            u = sp.tile([128, 512], F32, tag="u")
                            nc.vector.tensor_mul(u[:], rot[:], sinT[:, fs])
                            nc.vector.tensor_mul(dstT[cc][:, fs], raw[:], cosT[:, fs])
                            nc.vector.tensor_add(
                                dstT[cc][:, fs], dstT[cc][:, fs], u[:]
                            )

                # v natural, packed [128, 4*65] with a ones column per head
                for pi in range(NQ):
                    ps_v = psv.tile([128, DC], F32, tag="ps_v")
                    for k in range(KC):
                        nc.tensor.matmul(
                            ps_v[:],
                            xT[k][:, pi * 128:(pi + 1) * 128],
                            wv_sb[k][:],
                            start=(k == 0),
                            stop=(k == KC - 1),
                        )
                    for h in range(HPC):
                        nc.vector.tensor_copy(
                            v_sb[pi][:, h * VW:h * VW + HD],
                            ps_v[:, h * HD:(h + 1) * HD],
                        )
                        nc.vector.memset(
                            v_sb[pi][:, h * VW + HD:h * VW + HD + 1], 1.0
                        )

            # ---------------- attention ----------------
            with (
                tc.tile_pool(name="at_m", bufs=3) as mp,
                tc.tile_pool(name="at_p", bufs=3) as ep,
                tc.tile_pool(name="at_pt", bufs=4) as tp,
                tc.tile_pool(name="at_sc", bufs=4) as scp,
                tc.tile_pool(name="ps_s", bufs=2, space="PSUM") as pss,
                tc.tile_pool(name="ps_o", bufs=2, space="PSUM") as pso,
                tc.tile_pool(name="ps_t", bufs=2, space="PSUM") as pst,
                tc.tile_pool(name="ps_ot", bufs=2, space="PSUM") as psot,
            ):
                for qi in range(NQ):
                    w = cnts[qi] * QT
                    nbank = (cnts[qi] + 3) // 4
                    mk = mp.tile([128, maxw], F32, tag="mk")
                    nc.sync.dma_start(mk[:, :w], mk_d[qi, :, :w])
                    opair = [None] * CC
                    for h in range(HPC):
                        cc, ho = h // 2, (h % 2) * 64
                        qs = slice(qi * 128, (qi + 1) * 128)
                        ps_b = [
                            pss.tile([128, 512], F32, tag="ps_s", name="ps_s")
                            for _ in range(nbank)
                        ]
                        for slot, kt0, ln in segs[qi]:
                            off = (slot % 4) * 128
                            nc.tensor.matmul(
                                ps_b[slot // 4][:, off:off + ln * 128],
                                qrT[cc][ho:ho + 64, qs],
                                krT[cc][ho:ho + 64, kt0 * 128:(kt0 + ln) * 128],
                                start=True,
                                stop=True,
                            )
                        pe = ep.tile([128, maxw], F32, tag="pe")
                        for bi in range(nbank):
                            wb = min(w - bi * 512, 512)
                            nc.scalar.activation(
                                pe[:, bi * 512:bi * 512 + wb],
                                ps_b[bi][:, :wb],
                                mybir.ActivationFunctionType.Exp,
                                bias=0.0,
                                scale=SCALE,
                            )
                        nc.vector.tensor_mul(pe[:, :w], pe[:, :w], mk[:, :w])
                        po = pso.tile([128, VW], F32, tag="po")
                        for j, kt in enumerate(kts[qi]):
                            pt_ps = pst.tile([128, 128], F32, tag="pt_ps")
                            nc.tensor.transpose(
                                pt_ps[:], pe[:, j * 128:(j + 1) * 128], ident[:]
                            )
                            pt = tp.tile([128, 128], F32, tag="pt")
                            nc.vector.tensor_copy(pt[:], pt_ps[:])
                            nc.tensor.matmul(
                                po[:],
                                pt[:],
                                v_sb[kt][:, h * VW:(h + 1) * VW],
                                start=(j == 0),
                                stop=(j == cnts[qi] - 1),
                            )
                        r = scp.tile([128, 1], F32, tag="r")
                        nc.vector.reciprocal(r[:], po[:, HD:HD + 1])
                        if opair[cc] is None:
                            opair[cc] = scp.tile(
                                [128, 128], F32, tag="opair", name="opair"
                            )
                        nc.vector.tensor_scalar_mul(
                            opair[cc][:, ho:ho + 64], po[:, :HD], r[:]
                        )
                    for cc in range(CC):
                        ot_ps = psot.tile([128, 128], F32, tag="ot_ps")
                        nc.tensor.transpose(ot_ps[:], opair[cc][:], ident[:])
                        nc.vector.tensor_copy(
                            otT[cc][:, qi * 128:(qi + 1) * 128], ot_ps[:]
                        )

            # ---------------- output projection ----------------
            with (
                tc.tile_pool(name="wo_sc", bufs=3) as wsc,
                tc.tile_pool(name="wo_ps", bufs=2, space="PSUM") as wps,
            ):
                for qi in range(NQ):
                    ob = wsc.tile([128, D], F32, tag="ob")
                    for n2 in range(2):
                        pw = wps.tile([128, 512], F32, tag="pw")
                        for cc in range(CC):
                            nc.tensor.matmul(
                                pw[:],
                                otT[cc][:, qi * 128:(qi + 1) * 128],
                                wo_sb[cc][:, n2 * 512:(n2 + 1) * 512],
                                start=(cc == 0),
                                stop=(cc == CC - 1),
                            )
                        nc.vector.tensor_copy(
                            ob[:, n2 * 512:(n2 + 1) * 512], pw[:]
                        )
                    nc.sync.dma_start(out_d[qi * 128:(qi + 1) * 128, :], ob[:])

    nc.compile()
    return nc


def _host_inputs(x, freqs_cos, freqs_sin, position_ids, mask01, kts, cnts, maxw,
                 Wq, Wk, Wv, Wo):
    """Per-core input maps."""
    in_maps = []
    r64 = np.zeros((HD, HD), np.float32)
    for i in range(HD // 2):
        r64[2 * i, 2 * i + 1] = -1.0
        r64[2 * i + 1, 2 * i] = 1.0
    r128 = np.zeros((128, 128), np.float32)
    r128[:64, :64] = r64
    r128[64:, 64:] = r64
    rT = np.ascontiguousarray(r128.T)
    ident = np.eye(128, dtype=np.float32)

    maskc = np.zeros((NQ, QT, maxw), np.float32)
    for qi in range(NQ):
        for j, kt in enumerate(kts[qi]):
            maskc[qi, :, j * QT:(j + 1) * QT] = mask01[
                qi * QT:(qi + 1) * QT, kt * QT:(kt + 1) * QT
            ]

    for c in range(NCORES):
        b, g = c // HG, c % HG
        pos = np.clip(position_ids[b].astype(np.int64), 0, freqs_cos.shape[0] - 1)
        cos_g = np.asarray(freqs_cos)[pos]  # [S, 32]
        sin_g = np.asarray(freqs_sin)[pos]
        cosT64 = np.repeat(cos_g.T, 2, axis=0).astype(np.float32)  # [64, S]
        sinT64 = np.repeat(sin_g.T, 2, axis=0).astype(np.float32)
        cs = slice(g * DC, (g + 1) * DC)
        in_maps.append({
            "xT": np.ascontiguousarray(x[b].T).astype(np.float32),
            "wq": np.ascontiguousarray(Wq[:, cs]).astype(np.float32),
            "wk": np.ascontiguousarray(Wk[:, cs]).astype(np.float32),
            "wv": np.ascontiguousarray(Wv[:, cs]).astype(np.float32),
            "wo": np.ascontiguousarray(Wo[cs, :]).astype(np.float32),
            "cosT": np.concatenate([cosT64, cosT64], axis=0),
            "sinT": np.concatenate([sinT64, sinT64], axis=0),
            "rT": rT,
            "ident": ident,
            "maskc": maskc,
        })
    return in_maps


_CACHE = {}


def _get_nc(mask_key, kts, cnts, segs, maxw):
    if mask_key not in _CACHE:
        _CACHE[mask_key] = _build_nc(kts, cnts, segs, maxw)
    return _CACHE[mask_key]


def kernel(x, freqs_cos, freqs_sin, position_ids, bigbird_mask, Wq, Wk, Wv, Wo,
           _want_results=False, _trace=False, **trace_kwargs):
    x = np.asarray(x)
    mask = np.asarray(bigbird_mask).astype(bool)
    kts, cnts, segs, maxw = _mask_pattern(mask)
    nc = _get_nc(mask.tobytes(), kts, cnts, segs, maxw)
    in_maps = _host_inputs(
        x, np.asarray(freqs_cos), np.asarray(freqs_sin), np.asarray(position_ids),
        mask.astype(np.float32), kts, cnts, maxw,
        np.asarray(Wq), np.asarray(Wk), np.asarray(Wv), np.asarray(Wo),
    )
    res = bass_utils.run_bass_kernel_spmd(
        nc, in_maps, list(range(NCORES)), trace=_trace, **trace_kwargs
    )
    out = np.zeros((B, S, D), np.float32)
    for c in range(NCORES):
        out[c // HG] += res.results[c]["out"]
    if _want_results:
        return out, res
    return out
